# revision 1
# baseline (speedup 1.0000x reference)
"""Trainium2 Bass kernel for nn_MultiHeadAttention (B=2, S=2048, E=1024, H=16).

Sharding: 8 NeuronCores = data-parallel over the 2 batches x tensor-parallel
over the 16 heads in 4 groups of 4 heads (Wq/Wk/Wv split column-wise, Wo
row-wise).  Each core computes a full-[S, E] partial of its batch's output;
the host sums the 4 head-group partials per batch.

Per-core device algorithm (S.T orientation so exp(S.T) feeds P@V directly):
  Q.T/K.T[n, s] = (wT chunk).T @ xT chunk      (e-outer, chases input DMAs)
  V[s, n]       = (xvT chunk).T @ wvT chunk    stored as v_ext = [V_h | ones]
  S.T_h[k, q]   = (K_h.T chunk).T @ Q_h.T      row-packed head pairs (d=64)
  P.T           = exp(S.T / 8)                 one ACT op per (k, head pair)
  [O.T_h; sums] = (v_ext_h).T @ P.T_h          fused: PSUM rows 0-63 = O.T_h,
                                               rows 64-127 = rowsum broadcast
  O.Tn_h        = O.T_h * recip(sums)          recip shifted p64->p0 via DMA
  out[m, :]     = sum_h (oT_h chunk).T @ woT_h

dtypes: matmul inputs for the projections are bf16 (host pre-cast halves the
HBM traffic); everything SBUF-internal (Q.T/K.T/V/P.T) is float32r (full
fp32 bits, reduced-precision multiply); accumulation is always fp32.
"""

import numpy as np
from contextlib import ExitStack

import ml_dtypes

import concourse.bass as bass
import concourse.mybir as mybir
import concourse.tile as tile
from concourse.tile import ScopedClock
from concourse.bass_utils import run_bass_kernel_spmd

# ---------------------------------------------------------------------------
# Workarounds for the walrus build on this stack, which rejects more than ONE
# semaphore wait per instruction ("Too many sync wait commands").
# ---------------------------------------------------------------------------
_orig_commit_instruction = tile.TileContext._commit_instruction


def _commit_instruction(self, inst, lazy_reg_writes=True):
    si = getattr(inst, "sync_info", None)
    if si is not None and si.on_wait and len(si.on_wait) > 1:
        waits = list(si.on_wait)
        for w in waits[:-1]:
            nop = mybir.InstNoOp(
                name=self.nc.get_next_instruction_name(),
                ins=[], outs=[], engine=inst.engine,
            )
            nop.bass_nofuse = True
            nop.sync_info = mybir.SyncInfo(on_wait=[w], on_update=[])
            _orig_commit_instruction(self, nop, lazy_reg_writes=False)
        inst.sync_info = mybir.SyncInfo(
            on_wait=[waits[-1]], on_update=list(si.on_update or [])
        )
    return _orig_commit_instruction(self, inst, lazy_reg_writes)


def _drain_and_barrier(self, tick_clock, wait_clock):
    nc = self.nc
    drain_inst = nc.sync.drain()
    wait_clock.add_sem_waits(
        drain_inst.ins, ScopedClock({None: tick_clock.global_clock})
    )
    si = drain_inst.ins.sync_info
    waits = list(si.on_wait) if si and si.on_wait else []
    if len(waits) > 1:
        drain_inst.ins.sync_info = mybir.SyncInfo(
            on_wait=waits[:1], on_update=list(si.on_update or [])
        )
        for w in waits[1:]:
            extra = nc.sync.drain()
            esi = extra.ins.sync_info
            extra.ins.sync_info = mybir.SyncInfo(
                on_wait=[w],
                on_update=list(esi.on_update or []) if esi else [],
            )
    nc.all_engine_barrier()
    assert self.sems is not None
    popped = nc._tile_sem_poison_stack.pop()
    assert popped is self._sem_poison
    nc.clear_and_free_semaphores(list(self.sems.allocated().values()))
    nc.all_engine_barrier()


def _apply_tilefix():
    tile.TileContext._commit_instruction = _commit_instruction
    tile.TileContext._drain_and_barrier = _drain_and_barrier


_apply_tilefix()

# ---------------------------------------------------------------------------
# Problem constants (hardcoded)
# ---------------------------------------------------------------------------
B, S, E, H = 2, 2048, 1024, 16
HC, D = 4, 64              # heads per core, head dim
NCORES = 8
NE = E // 128              # 8  e-chunks
NQ = S // 512              # 4  q-chunks
NK = S // 128              # 16 k-chunks
NM = S // 128              # 16 m-chunks

F32 = mybir.dt.float32
BF16 = mybir.dt.bfloat16


def build(mmdt=mybir.dt.float32r, pdt=mybir.dt.float32r, xdt=BF16,
          ovbufs=3, xbufs=1, shift_eng="gpsimd"):
    nc = bass.Bass()
    xqT = nc.dram_tensor("xqT", [E, S], xdt, kind="ExternalInput")
    xkT = nc.dram_tensor("xkT", [E, S], xdt, kind="ExternalInput")
    xvT = nc.dram_tensor("xvT", [E, S], xdt, kind="ExternalInput")
    wqT = nc.dram_tensor("wqT", [E, 256], xdt, kind="ExternalInput")
    wkT = nc.dram_tensor("wkT", [E, 256], xdt, kind="ExternalInput")
    wvT = nc.dram_tensor("wvT", [E, 256], xdt, kind="ExternalInput")
    woT = nc.dram_tensor("woT", [256, E], BF16, kind="ExternalInput")
    vones = nc.dram_tensor("vones", [128, 256], mmdt, kind="ExternalInput")
    out = nc.dram_tensor("out", [S, E], F32, kind="ExternalOutput")

    with tile.TileContext(nc) as tc, ExitStack() as ctx:
        consts = ctx.enter_context(tc.tile_pool(name="consts", bufs=1))
        wpool = ctx.enter_context(tc.tile_pool(name="w", bufs=1))
        actpool = ctx.enter_context(tc.tile_pool(name="acts", bufs=1))
        xpool = ctx.enter_context(tc.tile_pool(name="x", bufs=10))

        # preload the exp table before the hot loop
        dummy = consts.tile([1, 8], F32)
        nc.vector.memset(dummy[:], 0.0)
        nc.scalar.activation(dummy[:], dummy[:], mybir.ActivationFunctionType.Exp)

        wv_sb = wpool.tile([128, NE, 256], xdt)
        wo_sb = wpool.tile([64, HC, E], BF16)

        qT_sb = actpool.tile([128, 2, S], mmdt)        # [(2 heads x d), pair, s]
        kT_sb = actpool.tile([128, 2, S], mmdt)
        v_sb = actpool.tile([128, NK, HC, 128], mmdt)  # [s%128, k, h, V_h|ones]

        def proj_eouter(w_sb, xchunks, dst, psA):
            tiles = [psA.tile([128, 512], F32, tag="mm", name=f"pj{i}")
                     for i in range(8)]
            for e in range(NE):
                for nch in range(2):
                    for m in range(NQ):
                        nc.tensor.matmul(
                            tiles[nch * NQ + m][:],
                            w_sb[:, e, nch * 128:(nch + 1) * 128],
                            xchunks[e][:, m * 512:(m + 1) * 512],
                            start=(e == 0), stop=(e == NE - 1),
                        )
            for nch in range(2):
                for m in range(NQ):
                    nc.vector.tensor_copy(
                        dst[:, nch, m * 512:(m + 1) * 512],
                        tiles[nch * NQ + m][:])

        # ---- prefix: K then Q projections (e-outer, DMA-chasing) ----
        with tc.tile_pool(name="wprefix", bufs=1) as wprefix, \
             tc.tile_pool(name="psA", bufs=8, space="PSUM") as psA:
            wk_sb = wprefix.tile([128, NE, 256], xdt)
            wq_sb = wprefix.tile([128, NE, 256], xdt)
            nc.sync.dma_start(wk_sb[:], wkT.rearrange("(ec p) n -> p ec n", p=128))
            nc.sync.dma_start(wq_sb[:], wqT.rearrange("(ec p) n -> p ec n", p=128))

            xk = []
            for e in range(NE):
                t = xpool.tile([128, S], xdt, tag="xchunk", name=f"xk{e}")
                nc.sync.dma_start(t[:], xkT[e * 128:(e + 1) * 128, :])
                xk.append(t)
            xq = []
            for e in range(NE):
                t = xpool.tile([128, S], xdt, tag="xchunk", name=f"xq{e}")
                nc.sync.dma_start(t[:], xqT[e * 128:(e + 1) * 128, :])
                xq.append(t)

            proj_eouter(wk_sb, xk, kT_sb, psA)
            proj_eouter(wq_sb, xq, qT_sb, psA)

        # V-side loads stream in behind the prefix on the SP queue
        nc.sync.dma_start(wv_sb[:], wvT.rearrange("(ec p) n -> p ec n", p=128))
        nc.sync.dma_start(wo_sb[:], woT.rearrange("(h p) j -> p h j", p=64))
        for k in range(NK):
            nc.gpsimd.dma_start(
                v_sb[:, k, :, 64:128],
                vones.rearrange("p (h c) -> p h c", h=HC))
        xv = []
        for e in range(NE):
            t = xpool.tile([128, S], xdt, tag="xchunk", name=f"xv{e}")
            nc.sync.dma_start(t[:], xvT[e * 128:(e + 1) * 128, :])
            xv.append(t)

        # ---- steady state pools ----
        oTpool = ctx.enter_context(tc.tile_pool(name="oT", bufs=1))
        ppool = ctx.enter_context(tc.tile_pool(name="pT", bufs=4))
        rpool = ctx.enter_context(tc.tile_pool(name="recip", bufs=2))
        opool = ctx.enter_context(tc.tile_pool(name="outstage", bufs=2))
        psS = ctx.enter_context(tc.tile_pool(name="psS", bufs=2, space="PSUM"))
        psOV = ctx.enter_context(tc.tile_pool(name="psOV", bufs=ovbufs, space="PSUM"))
        psX = ctx.enter_context(tc.tile_pool(name="psX", bufs=xbufs, space="PSUM"))

        oT_sb = oTpool.tile([64, HC, S], BF16)         # [d, h, s]

        def v_proj_tile(m):
            ps = psX.tile([128, 512], F32, tag="px", name=f"vp{m}")
            for e in range(NE):
                nc.tensor.matmul(
                    ps[:, 0:256],
                    xv[e][:, m * 128:(m + 1) * 128],
                    wv_sb[:, e, :],
                    start=(e == 0), stop=(e == NE - 1),
                )
            nc.vector.tensor_copy(
                v_sb[:, m, :, 0:64],
                ps[:, 0:256].rearrange("p (h c) -> p h c", h=HC))

        def out_proj_tile(m):
            stage = opool.tile([128, E], F32)
            for j in range(2):
                ps = psX.tile([128, 512], F32, tag="px", name=f"op{m}_{j}")
                for h in range(HC):
                    nc.tensor.matmul(
                        ps[:],
                        oT_sb[:, h, m * 128:(m + 1) * 128],
                        wo_sb[:, h, j * 512:(j + 1) * 512],
                        start=(h == 0), stop=(h == HC - 1),
                    )
                nc.vector.tensor_copy(stage[:, j * 512:(j + 1) * 512], ps[:])
            nc.gpsimd.dma_start(out[m * 128:(m + 1) * 128, :], stage[:])

        # V tiles are needed from the very first pass: emit them first
        for m in range(NM):
            v_proj_tile(m)

        for qc in range(NQ):
            qs = slice(qc * 512, (qc + 1) * 512)
            for pair in range(2):
                ps_ov = [psOV.tile([128, 512], F32, name=f"ov{i}", tag="ov")
                         for i in range(2)]
                for k in range(NK):
                    ks = slice(k * 128, (k + 1) * 128)
                    first, last = (k == 0), (k == NK - 1)
                    ps_s = psS.tile([128, 1024], F32)
                    # scores, row-packed: head A rows 0-63, head B rows 64-127
                    nc.tensor.matmul(ps_s[:, 0:512],
                                     kT_sb[0:64, pair, ks],
                                     qT_sb[0:64, pair, qs],
                                     start=True, stop=True)
                    nc.tensor.matmul(ps_s[:, 512:1024],
                                     kT_sb[64:128, pair, ks],
                                     qT_sb[64:128, pair, qs],
                                     start=True, stop=True)
                    # exp of both heads in one op; 1/sqrt(D) folded into scale
                    pT = ppool.tile([128, 1024], pdt)
                    nc.scalar.activation(pT[:], ps_s[:],
                                         mybir.ActivationFunctionType.Exp,
                                         scale=0.125)
                    # fused O.T + rowsum accumulation per head
                    for h2 in range(2):
                        h = pair * 2 + h2
                        nc.tensor.matmul(
                            ps_ov[h2][:],
                            v_sb[:, k, h, :],
                            pT[:, h2 * 512:(h2 + 1) * 512],
                            start=first, stop=last)
                # normalize: recip of sums (rows 64-127), shift to rows 0-63
                for h2 in range(2):
                    h = pair * 2 + h2
                    rt = rpool.tile([128, 512], F32, tag="rt")
                    nc.vector.reciprocal(rt[64:128, :], ps_ov[h2][64:128, :])
                    rb = rpool.tile([64, 512], F32, tag="rb")
                    getattr(nc, shift_eng).dma_start(rb[:], rt[64:128, :])
                    nc.vector.tensor_tensor(
                        oT_sb[:, h, qs], ps_ov[h2][0:64, :], rb[:],
                        mybir.AluOpType.mult)
            # out-proj for this q window (needs both pairs of this qc)
            for m in range(qc * 4, qc * 4 + 4):
                out_proj_tile(m)

    return nc


_NC_CACHE = {}


def _get_nc():
    if "nc" not in _NC_CACHE:
        _NC_CACHE["nc"] = build()
    return _NC_CACHE["nc"]


def _shard_inputs(query, key, value, Wq, Wk, Wv, Wo):
    """Host-side sharding + layout prep: core c = (batch c//4, head-group c%4)."""
    bf = ml_dtypes.bfloat16
    xT = []
    for b in range(B):
        xT.append((
            np.ascontiguousarray(query[b].T).astype(bf),
            np.ascontiguousarray(key[b].T).astype(bf),
            np.ascontiguousarray(value[b].T).astype(bf),
        ))
    wT = []
    for g in range(4):
        gc = slice(g * 256, (g + 1) * 256)
        wT.append((
            np.ascontiguousarray(Wq[gc].T).astype(bf),
            np.ascontiguousarray(Wk[gc].T).astype(bf),
            np.ascontiguousarray(Wv[gc].T).astype(bf),
            np.ascontiguousarray(Wo[:, gc].T).astype(bf),
        ))
    vones = np.ones((128, 256), dtype=np.float32)
    in_maps = []
    for c in range(NCORES):
        b, g = c // 4, c % 4
        qT, kT, vT = xT[b]
        wq, wk, wv, wo = wT[g]
        in_maps.append({
            "xqT": qT, "xkT": kT, "xvT": vT,
            "wqT": wq, "wkT": wk, "wvT": wv, "woT": wo,
            "vones": vones,
        })
    return in_maps


def kernel(query, key, value, Wq, Wk, Wv, Wo):
    query = np.asarray(query, dtype=np.float32)
    key = np.asarray(key, dtype=np.float32)
    value = np.asarray(value, dtype=np.float32)
    Wq = np.asarray(Wq, dtype=np.float32)
    Wk = np.asarray(Wk, dtype=np.float32)
    Wv = np.asarray(Wv, dtype=np.float32)
    Wo = np.asarray(Wo, dtype=np.float32)

    nc = _get_nc()
    in_maps = _shard_inputs(query, key, value, Wq, Wk, Wv, Wo)
    res = run_bass_kernel_spmd(nc, in_maps, core_ids=list(range(NCORES)))

    out = np.zeros((B, S, E), dtype=np.float32)
    for c in range(NCORES):
        out[c // 4] += res.results[c]["out"]
    return out
